# revision 29
# baseline (speedup 1.0000x reference)
"""BiDAF attention-flow kernel for Trainium2 (Bass/Tile), SPMD over 8 cores.

Math (per batch element b, one NeuronCore each):
    cq[c,j] = sum_h e2[c,h] * wcq[h] * e1[j,h]
    s[c,j]  = sc[c] + sq[j] + cq[c,j]            (+ scalar biases, which
                                                  cancel in both softmaxes)
    a       = softmax_j(s)
    c2q     = a @ e1                              (B,C,H)
    b_att   = softmax_c(max_j s)
    q2c     = b_att @ e2                          (H,)
    out     = [e2, c2q, e2*c2q, e2*q2c] @ w_red.T + b_red

Device layout: everything lives transposed, [h on partitions, c free]:
    sT[j,c] (PSUM) -> P_T = exp(sT + sq[j])      (unnormalized; row max not
                                                  subtracted - fp32 range is
                                                  plenty for |s| <= ~12)
    L[c] = sum_j P_T  via ones-matmul            a = P_T / L
    c2qT[h,c] = e1.T @ P_T, scaled by 1/L at PSUM eviction
    max_j s   = partition_all_reduce-max of max-over-jt-tiles of P_T
                (exp is monotone), so E = M*exp(sc) with no transposes, and
    q2c = (sum_c E[c]*e2T[:,c]) / sum_c E[c]     accumulated unnormalized
    q2c folded into the weights: wsum[h,:] = wrT[ht] + q2c[h]*wrT[18+ht]

All DRAM inputs are host-packed to the exact on-chip layout (partition
dim first, contiguous per-partition rows) so every load is one maximal
contiguous DMA segment per partition - the naive rearranged patterns
run at ~1/3 of DMA peak on 1KB segments.

Schedule: chunk-outer over c (4 chunks of 512) so each chunk's
cross-partition max all-reduce (GpSimd, full contiguous tiles - the
ucode mishandles sliced APs) fires as soon as that chunk's scores are
done.  E = M*exp(sc), the S row-sums and the q2c mul-reduce batches run
on DVE delayed by one chunk, so they never head-of-line-block the next
chunk's running maxes (Tile's coalesced cross-engine counters otherwise
serialize the all-reduces against them).  The folded weight tail is
ready near the end of phase B, and the reduction layer runs fully fused
(single PSUM accumulation of 18 k-tiles) for every chunk with the bias
added at eviction from a broadcast tile.  The e2*c2q products split:
chunk 0 on DVE right behind its evictions, chunk 1 on GpSimd during
phase B, chunks 2-3 on DVE at the start of the reduction.

Host does sharding/layout only: batch split, transposes, bf16 casts.
"""

import numpy as np
import ml_dtypes

B, Q, C, H, OUT = 8, 512, 2048, 768, 300
HT, JT, CT = H // 128, Q // 128, C // 128  # 6, 4, 16
NCH, CHW = 4, 512  # c chunks
CPT = 4  # c-tiles per chunk

bf16 = ml_dtypes.bfloat16

_CACHE = {}


def _build_bass():
    import concourse.tile as tile
    from concourse import mybir, bass_isa, library_config, bacc

    f32 = mybir.dt.float32
    b16 = mybir.dt.bfloat16
    AF = mybir.ActivationFunctionType

    nc = bacc.Bacc("TRN2", target_bir_lowering=False, debug=False)

    e1_d = nc.dram_tensor("e1", [128, JT, H], b16, kind="ExternalInput").ap()
    e1t_d = nc.dram_tensor("e1t", [128, HT, Q], b16, kind="ExternalInput").ap()
    e2t_d = nc.dram_tensor(
        "e2t", [128, NCH, HT, CHW], b16, kind="ExternalInput"
    ).ap()
    wrt_d = nc.dram_tensor(
        "wrt", [128, 24, OUT], b16, kind="ExternalInput"
    ).ap()
    wpk_d = nc.dram_tensor("wpk", [128, 3 * HT], f32, kind="ExternalInput").ap()
    bred_d = nc.dram_tensor("bred", [1, OUT], b16, kind="ExternalInput").ap()
    out_d = nc.dram_tensor("out", [C, OUT], f32, kind="ExternalOutput").ap()

    with tile.TileContext(nc) as tc:
        with (
            tc.tile_pool(name="singles", bufs=1) as singles,
            tc.tile_pool(name="m3", bufs=8) as m3p,
            tc.tile_pool(name="odma", bufs=4) as odp,
            tc.tile_pool(name="ps_mm", bufs=6, space="PSUM") as ps_mm,
            tc.tile_pool(name="ps_out", bufs=2, space="PSUM") as ps_out,
        ):
            nc.gpsimd.load_library(library_config.attn)

            # ---- persistent SBUF tensors -------------------------------
            e1_sb = singles.tile([128, JT, H], b16)      # emb1, j on parts
            e1t_sb = singles.tile([128, HT, Q], b16)     # emb1.T, h on parts
            e1w_sb = singles.tile([128, HT, Q], b16)     # wcq * emb1.T
            e2t_sb = singles.tile([128, NCH, HT, CHW], b16)  # emb2.T chunked
            wrt_sb = singles.tile([128, 24, OUT], b16)   # w_red.T, k on parts
            wq4_sb = singles.tile([128, HT, OUT], b16)   # q2c-folded wrT tail
            wsum_sb = singles.tile([128, HT, OUT], b16)  # wrT[0:6] + wq4T
            wpk_sb = singles.tile([128, 3 * HT], f32)
            wq_sb = singles.tile([128, HT], b16)
            bredb_sb = singles.tile([128, OUT], f32)     # b_red bcast
            bred_sb = singles.tile([1, OUT], b16)
            ones_mat = singles.tile([128, 128], b16)
            ones_row_b = singles.tile([1, 128], b16)
            sq_sb = singles.tile([128, JT], f32)         # sq as columns
            escb_sb = singles.tile([128, C], b16)        # exp(sc) bcast
            wc_mat = singles.tile([128, HT, 128], b16)   # wc[h] rank-1 bcast
            pt_sb = singles.tile([128, JT, NCH, CHW], b16)  # P_T = exp(sT+sq)
            c2q_sb = singles.tile([128, HT, C], b16)     # c2qT (normalized)
            m3_sb = singles.tile([128, HT, C], b16)      # e2*c2q, transposed
            macc_c = [
                singles.tile([128, CHW], b16, name=f"macc{i}")
                for i in range(NCH)
            ]
            mall_c = [
                singles.tile([128, CHW], b16, name=f"mall{i}")
                for i in range(NCH)
            ]
            ebc_sb = singles.tile([128, C], b16)         # E bcast over parts
            s_parts = singles.tile([128, NCH], f32)
            s_sum = singles.tile([128, 1], f32)
            rs_col = singles.tile([128, 1], f32)
            q2cn_sb = singles.tile([128, HT], f32)
            bcr_sb = singles.tile([128, C], f32)         # 1/L bcast over parts
            u_sb = singles.tile([128, HT, NCH], f32)     # unnormalized q2c
            q2c_sb = singles.tile([128, HT], f32)

            # ---- DMA issue: 3 queues so transfers overlap --------------
            # sync: e2t chunk 0 -> e1 -> wrt; scalar: e1t -> e2t chunks
            # 1-3; vector: the small weights.  The two big critical-path
            # transfers (e1t, e2t chunk 0) run concurrently.
            nc.scalar.dma_start(out=wpk_sb, in_=wpk_d)
            nc.scalar.dma_start(out=e1t_sb, in_=e1t_d)
            nc.sync.dma_start(out=e2t_sb[:, 0], in_=e2t_d[:, 0])
            nc.sync.dma_start(out=bred_sb, in_=bred_d)
            for ch in range(1, NCH):
                nc.scalar.dma_start(out=e2t_sb[:, ch], in_=e2t_d[:, ch])
            nc.sync.dma_start(out=e1_sb, in_=e1_d)
            nc.sync.dma_start(out=wrt_sb, in_=wrt_d)

            # ---- host-free constants on DVE ----------------------------
            nc.vector.memset(ones_mat, 1.0)
            nc.vector.memset(ones_row_b, 1.0)
            wcq_sb = wpk_sb[:, 0:HT]
            nc.vector.tensor_copy(wq_sb, wpk_sb[:, 2 * HT : 3 * HT])
            for ht in range(HT):
                nc.vector.tensor_scalar_mul(
                    wc_mat[:, ht, :], ones_mat,
                    wpk_sb[:, HT + ht : HT + ht + 1],
                )
            for ht in range(HT):
                nc.vector.tensor_scalar_mul(
                    e1w_sb[:, ht, :], e1t_sb[:, ht, :], wcq_sb[:, ht : ht + 1]
                )

            # ---- PE: warm-up (p-state ramp while inputs stream in) -----
            wps = ps_mm.tile([128, CHW], f32, tag="mm", name="warm")
            for _ in range(34):
                nc.tensor.matmul(wps[:, 0:128], ones_mat, ones_mat,
                                 start=True, stop=True)

            # sq columns (tiny, feeds exp bias); evicted on ACT
            for jt in range(JT):
                ps = ps_mm.tile([128, CHW], f32, tag="mm")
                for ht in range(HT):
                    nc.tensor.matmul(
                        ps[:, 0:1],
                        e1t_sb[:, ht, jt * 128 : (jt + 1) * 128],
                        wq_sb[:, ht : ht + 1],
                        start=(ht == 0),
                        stop=(ht == HT - 1),
                    )
                nc.scalar.copy(sq_sb[:, jt : jt + 1], ps[:, 0:1])

            for _ in range(2):
                nc.tensor.matmul(wps[:, 0:128], ones_mat, ones_mat,
                                 start=True, stop=True)

            def emit_sc(ch):
                # sc: rank-1 weights broadcast exp(sc) over all partitions
                csl = slice(ch * CHW, (ch + 1) * CHW)
                scps = ps_mm.tile([128, CHW], f32, tag="mm", name=f"scps{ch}")
                for ht in range(HT):
                    nc.tensor.matmul(
                        scps, wc_mat[:, ht, :], e2t_sb[:, ch, ht, :],
                        start=(ht == 0), stop=(ht == HT - 1),
                    )
                nc.scalar.activation(
                    out=escb_sb[:, csl], in_=scps, func=AF.Exp,
                    bias=0.0, scale=1.0,
                )

            def emit_amr(ch):
                # E row + q2c mul-reduce batch for chunk ch on DVE (the
                # all-reduce for ch is long done by the time DVE gets
                # here); S partial on every partition (rows identical)
                csl = slice(ch * CHW, (ch + 1) * CHW)
                nc.vector.tensor_mul(
                    ebc_sb[:, csl], mall_c[ch], escb_sb[:, csl]
                )
                nc.vector.reduce_sum(
                    out=s_parts[:, ch : ch + 1], in_=ebc_sb[:, csl],
                    axis=mybir.AxisListType.X,
                )
                for ht in range(HT):
                    am = m3p.tile([128, CHW], b16, tag="m3",
                                  name=f"am{ch}_{ht}")
                    nc.vector.affine_mul_reduce(
                        out=am,
                        accum_out=u_sb[:, ht, ch : ch + 1],
                        in0=e2t_sb[:, ch, ht, :],
                        in1=ebc_sb[:, csl],
                        scale=1.0,
                        bias=0.0,
                    )

            emit_sc(0)

            # ---- phase A, chunk-outer ----------------------------------
            # per chunk: scores (4 jt-groups of 6 ht-accumulated matmuls)
            # + exp (bias sq) on ACT + running max on DVE; L ones-matmuls
            # + 1/L.  Cross-partition max on GpSimd per chunk; the
            # previous chunk's E/q2c batch runs on DVE one chunk late.
            for ch in range(NCH):
                csl = slice(ch * CHW, (ch + 1) * CHW)
                if ch > 0:
                    emit_sc(ch)
                for jt in range(JT):
                    sps = ps_mm.tile([128, CHW], f32, tag="mm",
                                     name=f"sps{ch}_{jt}")
                    for ht in range(HT):
                        nc.tensor.matmul(
                            sps,
                            e1w_sb[:, ht, jt * 128 : (jt + 1) * 128],
                            e2t_sb[:, ch, ht, :],
                            start=(ht == 0),
                            stop=(ht == HT - 1),
                        )
                    nc.scalar.activation(
                        out=pt_sb[:, jt, ch, :], in_=sps, func=AF.Exp,
                        bias=sq_sb[:, jt : jt + 1], scale=1.0,
                    )
                    if jt == 1:
                        nc.vector.tensor_max(
                            macc_c[ch], pt_sb[:, 0, ch, :],
                            pt_sb[:, 1, ch, :],
                        )
                    elif jt > 1:
                        nc.vector.tensor_max(
                            macc_c[ch], macc_c[ch], pt_sb[:, jt, ch, :]
                        )
                if ch == 0:
                    # b_red broadcast to all partitions, evicted on ACT
                    bps = ps_mm.tile([128, CHW], f32, tag="mm", name="bps")
                    nc.tensor.matmul(bps[:, 0:OUT], ones_row_b, bred_sb,
                                     start=True, stop=True)
                    nc.scalar.copy(bredb_sb, bps[:, 0:OUT])
                # L (replicated on all partitions via all-ones weights)
                lps = ps_mm.tile([128, CHW], f32, tag="mm", name=f"lps{ch}")
                for jt in range(JT):
                    nc.tensor.matmul(
                        lps, ones_mat, pt_sb[:, jt, ch, :],
                        start=(jt == 0), stop=(jt == JT - 1),
                    )
                nc.vector.reciprocal_approx_fast(out=bcr_sb[:, csl], in_=lps)
                # cross-partition max for this chunk on GpSimd
                nc.gpsimd.partition_all_reduce(
                    mall_c[ch], macc_c[ch], channels=128,
                    reduce_op=bass_isa.ReduceOp.max,
                )
                if ch > 0:
                    emit_amr(ch - 1)

            # ---- phase B: c2qT matmuls with 1/L eviction scaling -------
            # ht outer / jt mid / ch inner: e1 stationary tile reused
            # across the 4 chunks (one LDWEIGHTS per (ht, jt)).  All
            # evictions on DVE; the last chunk's E/q2c batch, the q2c
            # finalize + weight fold (scale muls on ACT) interleave
            # between eviction rounds.  m3 chunk 0 rides DVE right behind
            # its evictions; m3 chunk 1 forms on GpSimd.
            csl3 = slice(3 * CHW, 4 * CHW)
            amr3_per_ht = [0, 0, 2, 2, 2, 0]
            amr3_done = 0
            for ht in range(HT):
                cps = [
                    (ps_mm if i < 3 else ps_out).tile(
                        [128, CHW], f32, tag=("mm" if i < 3 else "out"),
                        name=f"cps{ht}_{i}")
                    for i in range(NCH)
                ]
                for jt in range(JT):
                    for ch in range(NCH):
                        nc.tensor.matmul(
                            cps[ch],
                            e1_sb[:, jt, ht * 128 : (ht + 1) * 128],
                            pt_sb[:, jt, ch, :],
                            start=(jt == 0),
                            stop=(jt == JT - 1),
                        )
                if ht == 5:
                    # wsum adds first (DVE would otherwise sit waiting on
                    # this ht's PSUM anyway) so the reduction never waits
                    for ht2 in range(HT):
                        nc.vector.tensor_add(
                            wsum_sb[:, ht2, :], wq4_sb[:, ht2, :],
                            wrt_sb[:, ht2, :],
                        )
                    nc.vector.tensor_mul(
                        c2q_sb[:, ht, 0:CHW], cps[0], bcr_sb[:, 0:CHW]
                    )
                    nc.vector.tensor_mul(
                        m3_sb[:, ht, 0:CHW], e2t_sb[:, 0, ht, :],
                        c2q_sb[:, ht, 0:CHW],
                    )
                    for ch in range(1, NCH):
                        csl = slice(ch * CHW, (ch + 1) * CHW)
                        nc.vector.tensor_mul(
                            c2q_sb[:, ht, csl], cps[ch], bcr_sb[:, csl]
                        )
                else:
                    for ch in range(NCH):
                        csl = slice(ch * CHW, (ch + 1) * CHW)
                        nc.vector.tensor_mul(
                            c2q_sb[:, ht, csl], cps[ch], bcr_sb[:, csl]
                        )
                    # m3 chunk 0 on GpSimd behind the eviction
                    nc.gpsimd.tensor_mul(
                        m3_sb[:, ht, 0:CHW], e2t_sb[:, 0, ht, :],
                        c2q_sb[:, ht, 0:CHW],
                    )
                # m3 chunk 1 on GpSimd (proven path, idle engine)
                nc.gpsimd.tensor_mul(
                    m3_sb[:, ht, CHW : 2 * CHW], e2t_sb[:, 1, ht, :],
                    c2q_sb[:, ht, CHW : 2 * CHW],
                )
                if ht == 1:
                    # last chunk's E row (all-reduce 3 done by now, off
                    # the ht0 round to avoid waiting on it)
                    nc.vector.tensor_mul(
                        ebc_sb[:, csl3], mall_c[3], escb_sb[:, csl3]
                    )
                elif ht == 2:
                    nc.vector.reduce_sum(
                        out=s_parts[:, 3:4], in_=ebc_sb[:, csl3],
                        axis=mybir.AxisListType.X,
                    )
                elif ht == 3:
                    nc.vector.reduce_sum(
                        out=s_sum, in_=s_parts, axis=mybir.AxisListType.X
                    )
                    nc.vector.reciprocal_approx_fast(out=rs_col, in_=s_sum)
                for _ in range(amr3_per_ht[ht]):
                    am = m3p.tile([128, CHW], b16, tag="m3",
                                  name=f"am3_{amr3_done}")
                    nc.vector.affine_mul_reduce(
                        out=am,
                        accum_out=u_sb[:, amr3_done, 3:4],
                        in0=e2t_sb[:, 3, amr3_done, :],
                        in1=ebc_sb[:, csl3],
                        scale=1.0,
                        bias=0.0,
                    )
                    amr3_done += 1
                if ht == 4:
                    # q2c finalize: q2c = U/S; scale muls on ACT
                    nc.vector.reduce_sum(
                        out=q2c_sb, in_=u_sb, axis=mybir.AxisListType.X
                    )
                    nc.vector.tensor_scalar_mul(q2cn_sb, q2c_sb, rs_col)
                    for ht2 in range(HT):
                        nc.scalar.activation(
                            out=wq4_sb[:, ht2, :],
                            in_=wrt_sb[:, 18 + ht2, :],
                            func=AF.Copy,
                            bias=0.0,
                            scale=q2cn_sb[:, ht2 : ht2 + 1],
                        )

            # ---- reduction layer: fully fused, all chunks --------------
            def emit_m3_dve(ch):
                csl = slice(ch * CHW, (ch + 1) * CHW)
                for ht in range(HT):
                    nc.vector.tensor_mul(
                        m3_sb[:, ht, csl], e2t_sb[:, ch, ht, :],
                        c2q_sb[:, ht, csl],
                    )

            emit_m3_dve(2)
            for ch in range(NCH):
                for lc in range(CPT):
                    ct = ch * CPT + lc
                    tsl = slice(lc * 128, (lc + 1) * 128)
                    lsl = slice(ch * CHW + lc * 128, ch * CHW + (lc + 1) * 128)
                    ops = ps_out.tile([128, OUT], f32, tag="out",
                                      name=f"ops{ct}")
                    for ht in range(HT):
                        nc.tensor.matmul(
                            ops, e2t_sb[:, ch, ht, tsl], wsum_sb[:, ht, :],
                            start=(ht == 0), stop=False,
                        )
                    for ht in range(HT):
                        nc.tensor.matmul(
                            ops, c2q_sb[:, ht, lsl], wrt_sb[:, 6 + ht, :],
                            start=False, stop=False,
                        )
                    for ht in range(HT):
                        nc.tensor.matmul(
                            ops, m3_sb[:, ht, lsl],
                            wrt_sb[:, 12 + ht, :],
                            start=False, stop=(ht == HT - 1),
                        )
                    od = odp.tile([128, OUT], f32, tag="od", name=f"od{ct}")
                    nc.vector.tensor_add(od, ops, bredb_sb)
                    nc.sync.dma_start(
                        out=out_d[ct * 128 : (ct + 1) * 128, :], in_=od
                    )
                    if ch == 0 and lc == 1:
                        emit_m3_dve(3)

    nc.compile()
    return nc


def _get_nc():
    if "nc" not in _CACHE:
        _CACHE["nc"] = _build_bass()
    return _CACHE["nc"]


def _in_maps(emb1, emb2, w_c, b_c, w_q, b_q, w_cq, b_cq, w_red, b_red):
    # host-side sharding + layout only: batch split, packed transposes to
    # the on-chip layout (partition dim first, contiguous rows), bf16
    emb1 = np.asarray(emb1, np.float32)
    emb2 = np.asarray(emb2, np.float32)
    wcq = np.asarray(w_cq, np.float32).reshape(HT, 128).T
    wc = np.asarray(w_c, np.float32).reshape(HT, 128).T
    wq = np.asarray(w_q, np.float32).reshape(HT, 128).T
    wpk = np.ascontiguousarray(np.concatenate([wcq, wc, wq], axis=1))
    wrt_flat = np.asarray(w_red, np.float32).T  # [4H, OUT]
    wrt = np.ascontiguousarray(
        wrt_flat.reshape(24, 128, OUT).transpose(1, 0, 2)
    ).astype(bf16)
    bred = np.asarray(b_red, np.float32).reshape(1, OUT).astype(bf16)
    maps = []
    for b in range(B):
        e1 = emb1[b]  # [Q, H]
        e2 = emb2[b]  # [C, H]
        e1p = np.ascontiguousarray(
            e1.reshape(JT, 128, H).transpose(1, 0, 2)
        ).astype(bf16)
        e1tp = np.ascontiguousarray(
            e1.T.reshape(HT, 128, Q).transpose(1, 0, 2)
        ).astype(bf16)
        e2tp = np.ascontiguousarray(
            e2.T.reshape(HT, 128, NCH, CHW).transpose(1, 2, 0, 3)
        ).astype(bf16)
        maps.append(
            {
                "e1": e1p,
                "e1t": e1tp,
                "e2t": e2tp,
                "wrt": wrt,
                "wpk": wpk,
                "bred": bred,
            }
        )
    return maps


def run(inputs, trace=False):
    from concourse.bass_utils import run_bass_kernel_spmd

    nc = _get_nc()
    maps = _in_maps(**inputs)
    res = run_bass_kernel_spmd(nc, maps, list(range(B)), trace=trace)
    out = np.stack([res.results[b]["out"] for b in range(B)], axis=0)
    return out.astype(np.float32), res


def kernel(**inputs) -> np.ndarray:
    out, _ = run(inputs, trace=False)
    return out
